# revision 15
# baseline (speedup 1.0000x reference)
"""Trainium2 Bass kernel for nn_ChannelModel (cross-attention + bilinear + logsigmoid sum).

Reference computation (full problem, N=16384, M=1024, Ds=2048):
    scores = (D @ S.T) / sqrt(Ds)            # [N, M]
    w      = softmax(scores, axis=1)         # [N, M]
    att_S  = w @ S                           # [N, Ds]
    logits[i] = D[i] . (W @ att_S[i])        # [N]
    out    = sum(log_sigmoid(logits))        # scalar

Algebraic restructuring:
    logits[i] = (sum_j e_ij * B[i,j]) / (sum_j e_ij)
    with  e = exp(scores/sqrt(Ds)),  B = D @ G.T,  G.T = W @ S.T
which removes the att_S matmul and the big bilinear matmul entirely.

Distribution over 8 cores: D row-sharded (2048 rows/core); S and W
replicated; each core computes G.T itself (no collectives by default;
see SHARD_A for the AllGather-sharded variant and why it is off).
Partial logsigmoid sums are added on the host.

Precision: all matmuls run fp8e4m3 inputs with DoubleRow (2 fp8 rows
per PE cell -> K=256 contraction per matmul instruction) and fp32 PSUM
accumulation. W is pre-scaled by 64 on the host so its entries (std
0.01) sit in fp8's normal range; the GT copy un-scales by 1/64. The
softmax exp and all reductions are fp32. Validated end-to-end error of
the final scalar vs the fp32 reference: ~7e-4 relative (threshold
2e-2) — the final sum averages 16384 logits, so elementwise fp8
quantization noise largely cancels.

Phases per core (emission order):
    B: scores + exp for all 16 row-tiles (256 DoubleRow matmuls),
       e cached in SBUF (bf16)
    A: GT = (64W) @ S.T / 64 (256 DoubleRow matmuls; 32 + AllGather
       when SHARD_A)
    C: B-matmul + e*B row-reduction (256 DoubleRow matmuls)
    epilogue: logits = lu/se; sum softplus(-logits) via an Exp-table
       Newton iteration (no Softplus/Ln ACT table exists on this build),
       first half emitted mid-phase-C on ACT/GpSimd so it overlaps.

Matmul scheduling notes (measured on this silicon):
    - K-outer / j-inner emission (alternating the two 512-wide PSUM
      half-banks every instruction) beats 8-deep same-bank chains by
      ~25% — the alternation hides PSUM accumulate RMW latency.
    - PSUM matmul slices must stay 512/bank-aligned: non-512 J splits
      both corrupt accumulation groups and run slower.
    - fp8 with and without DoubleRow cost the same per instruction, so
      DR's 2x contraction per instruction is pure win.
"""

import math
import os
import sys

for _p in ("/opt/trn_rl_repo", "/root/.axon_site/_ro/trn_rl_repo"):
    if os.path.isdir(_p) and _p not in sys.path:
        sys.path.insert(0, _p)

import ml_dtypes
import numpy as np

import concourse.bass as bass
import concourse.tile as tile
from concourse import bacc, mybir
from concourse.bass_utils import run_bass_kernel_spmd

N_CORES = 8
N_FULL = 16384
M = 1024
DS = 2048
N_LOC = N_FULL // N_CORES   # 2048 rows per core
NT = N_LOC // 128           # 16 row-tiles per core
KT = DS // 128              # 16 contraction slices
PT = KT // 2                # 8 DoubleRow contraction pair-slices
LS = KT // N_CORES          # 2 local G.T row-blocks per core when sharded

W_SCALE = 64.0              # host pre-scale of W for fp8 normal range

BF16 = mybir.dt.bfloat16
FP8 = mybir.dt.float8e4
F32 = mybir.dt.float32
DR = mybir.MatmulPerfMode.DoubleRow

# Benchmark knob: when set >1, the kernel body runs LOOP_R times inside one
# NEFF via a hardware loop (used by the timing harness only). The AllGather
# cannot execute inside a hardware loop, so loop builds prime it once before
# the loop; every iteration still performs the sharded phase-A compute, the
# DRAM src write and the 2MB gather-in reads — only the link transport
# (which overlaps phase B in the single-shot build) is outside the loop.
LOOP_R = None

# Shard phase A (G.T) across cores: cuts its PE work 8x (256 -> 32 matmul
# instructions/core). "rdma": gather the 8 G.T blocks with direct SBUF->SBUF
# remote_dma_broadcast sends (SDMA hardware path; transport overlaps phase B
# and runs inside the timing loop). True: gather via collective_compute
# through shared DRAM — correct but ~290us for 2MB on this runtime's
# emulated transport (measured), so only kept as a fallback. False: every
# core computes the full G.T from a replicated W (no communication).
SHARD_A = False

# STATUS of "rdma" (2026-08-08): machinery works end-to-end in a standalone
# 8-core test (work/rdma_test.py — slot data verified, including the
# relative-dest quirk below), and in-kernel the local slot and SOME slot-1
# deliveries land correctly, but the remaining sends stall (arrival sem never
# reaches 14 -> watchdog), failure mode unresolved — suspect SWDGE ring
# servicing contention with the kernel's other gpsimd work. Do NOT default
# to "rdma" until the standalone-vs-in-kernel delta is root-caused.
#
# remote_dma_broadcast relative-dest quirk (measured on this silicon): data
# sent with rdests[d]=(0,d) lands on peer (me ^ d ^ (2 if d&4 else 0)).
# Host-side per-core permutation of contraction blocks absorbs this so slot
# indices are compile-time uniform across cores.


def _peer_xor(d):
    return d ^ 2 if d & 4 else d


def _block_perm(c):
    """Global LS-slice block held at local slot d on core c."""
    return [c ^ _peer_xor(d) for d in range(N_CORES)]

J_SPLITS = (512, 512)       # PSUM-bank-aligned M-axis split (mandatory 512s)
GROUP = 1                   # output tiles interleaved per matmul burst


def _chunks(seq, g):
    seq = list(seq)
    return [seq[i:i + g] for i in range(0, len(seq), g)]


def _j_ranges():
    out, off = [], 0
    for w in J_SPLITS:
        out.append((off, off + w))
        off += w
    assert off == M
    return out


def _build_program():
    nc = bacc.Bacc("TRN2", target_bir_lowering=False, debug=False,
                   num_devices=N_CORES)

    # DRAM parameters (per-core shapes; packed on host, see kernel()).
    # dtp[it, p, ds, ii] = D_shard[it*128+ii, ds*128+p]
    dt_ap = nc.dram_tensor("dtp", [NT, 128, KT, 128], FP8,
                           kind="ExternalInput").ap()
    # stp8[p, es, j] = S[j, es*128+p]  (= S.T, Ds on partitions)
    st8_ap = nc.dram_tensor("stp8", [128, KT, M], FP8,
                            kind="ExternalInput").ap()
    # wtp[dt, p, es, ii] = 64*W[dt_glob*128+ii, es*128+p]; sharded builds
    # ship only this core's LS row-blocks (dt_glob = core*LS + dt).
    wt_slices = LS if SHARD_A else KT
    wt_ap = nc.dram_tensor("wtp", [wt_slices, 128, KT, 128], FP8,
                           kind="ExternalInput").ap()
    out_ap = nc.dram_tensor("out", [1, 1], F32, kind="ExternalOutput").ap()
    gtdbg_ap = (nc.dram_tensor("gtdbg", [128, KT, M], FP8,
                               kind="ExternalOutput").ap()
                if os.environ.get("RDMA_DEBUG") else None)

    if SHARD_A is True:
        # AllGather bounce buffers (CC fallback mode). The collective
        # concatenates along the partition dim: [128, LS*M] -> [8*128, LS*M].
        ag_src = nc.dram_tensor("agsrc", [128, LS * M], FP8)
        ag_dst = nc.dram_tensor("agdst", [N_CORES, 128, LS * M], FP8,
                                addr_space="Shared")
    if SHARD_A == "rdma":
        # Cross-core arrival semaphore (+2 per sender x 7) and send-complete
        # semaphore. SPMD-identical allocation pins the same sem num on all
        # cores, which remote_dma requires.
        gt_rsem = nc.alloc_semaphore("gt_rsem")
        gt_lsem = nc.alloc_semaphore("gt_lsem")
        rdma_gates = []

    scale = 1.0 / math.sqrt(DS)
    Exp = mybir.ActivationFunctionType.Exp
    Relu = mybir.ActivationFunctionType.Relu

    with tile.TileContext(nc) as tc:
        with (
            tc.tile_pool(name="singles", bufs=1) as singles,
            tc.tile_pool(name="wt_pool", bufs=2 * GROUP + 1) as wt_pool,
            tc.tile_pool(name="dt_pool", bufs=16) as dt_pool,
            tc.tile_pool(name="prod_pool", bufs=2 * GROUP + 1) as prod_pool,
            tc.tile_pool(name="psum", bufs=4, space="PSUM") as psum_pool,
        ):
            def _emit_phase_a_local(st8_sb, gt_loc, emit_cc):
                """Sharded phase A: LS local G.T row-blocks into gt_loc
                (slot 0 of gt8_sb in rdma mode), then the gather sends."""
                for li in range(LS):
                    wt_t = wt_pool.tile([128, KT, 128], FP8, tag="wt",
                                        name=f"wt{li}")
                    nc.sync.dma_start(out=wt_t[:], in_=wt_ap[li])
                    pg = psum_pool.tile([128, M], F32, tag="s")
                    for k in range(PT):
                        ks = slice(2 * k, 2 * k + 2)
                        for j0, j1 in _j_ranges():
                            js = slice(j0, j1)
                            nc.tensor.matmul(pg[:, js], wt_t[:, ks, :],
                                             st8_sb[:, ks, js], perf_mode=DR,
                                             start=(k == 0), stop=(k == PT - 1))
                    # un-scale by 1/64, quantize to fp8 in one DVE pass
                    nc.vector.tensor_scalar_mul(gt_loc[:, li, :], pg[:],
                                                1.0 / W_SCALE)
                if SHARD_A == "rdma":
                    return
                nc.scalar.dma_start(out=ag_src[:], in_=gt_loc[:])
                if emit_cc:
                    nc.gpsimd.collective_compute(
                        "AllGather", mybir.AluOpType.bypass,
                        ins=[ag_src[:]], outs=[ag_dst[:]],
                        replica_groups=[list(range(N_CORES))],
                    )

            def _emit_rdma_gather(gt8_sb):
                """Send my slot-0 block to each relative peer's slot d.
                Host-side block permutation makes slots uniform (see
                _block_perm). Descriptors + trigger run on gpsimd; the
                SDMA transport overlaps phase B."""
                for d in range(1, N_CORES):
                    rd = [None] * N_CORES
                    rd[d] = (0, d)
                    nc.gpsimd.remote_dma_broadcast(
                        out_ap=gt8_sb[:, d * LS:(d + 1) * LS, :],
                        in_ap=gt8_sb[:, 0:LS, :],
                        remote_sem=gt_rsem, local_sem=gt_lsem, rdests=rd,
                    )
                nc.gpsimd.trigger_dma(count=None)

            def _emit_body(emit_cc=True):
                # Long-lived SBUF tensors.
                st8_sb = singles.tile([128, KT, M], FP8)
                gt8_sb = singles.tile([128, KT, M], FP8)
                e_all = singles.tile([128, NT, M], BF16)
                se_buf = singles.tile([128, NT], F32)
                lu_buf = singles.tile([128, NT], F32)
                if SHARD_A == "rdma":
                    gt_loc = gt8_sb[:, 0:LS, :]
                elif SHARD_A:
                    gt_loc = singles.tile([128, LS, M], FP8)

                # st8 chunks fan out over three engines' DMA queues so the
                # 2MB load lands in ~5us instead of ~12us serial.
                _dma_engs = (nc.sync, nc.scalar, nc.gpsimd)
                for es in range(KT):
                    _dma_engs[es % 3].dma_start(out=st8_sb[:, es, :],
                                                in_=st8_ap[:, es, :])

                # ---- Phase A first: its gather transport overlaps B ----
                if SHARD_A:
                    _emit_phase_a_local(st8_sb, gt_loc, emit_cc)
                if SHARD_A == "rdma":
                    _emit_rdma_gather(gt8_sb)

                # ---- Phase B: scores + exp for all row-tiles (fp8 DR) ----
                # dt tiles stay resident for reuse in phase C.
                dts = []
                for chunk in _chunks(range(NT), GROUP):
                    group = []
                    for it in chunk:
                        dt_t = dt_pool.tile([128, KT, 128], FP8, tag="dt",
                                            name=f"dtb{it}")
                        nc.sync.dma_start(out=dt_t[:], in_=dt_ap[it])
                        dts.append(dt_t)
                        ps = psum_pool.tile([128, M], F32, tag="s")
                        group.append((it, dt_t, ps))
                    for k in range(PT):
                        ks = slice(2 * k, 2 * k + 2)
                        for it, dt_t, ps in group:
                            for j0, j1 in _j_ranges():
                                js = slice(j0, j1)
                                nc.tensor.matmul(ps[:, js], dt_t[:, ks, :],
                                                 st8_sb[:, ks, js],
                                                 perf_mode=DR, start=(k == 0),
                                                 stop=(k == PT - 1))
                    for it, dt_t, ps in group:
                        nc.scalar.activation(
                            out=e_all[:, it, :], in_=ps[:], func=Exp,
                            scale=scale, accum_out=se_buf[:, it:it + 1],
                        )

                # ---- G.T into SBUF ----
                if SHARD_A == "rdma":
                    # Arrival gate on the PE queue before any phase-C matmul
                    # touches a remote slot: 7 senders x 2 increments. Emitted
                    # as >=0 so the single-core scheduling sim (which cannot
                    # see remote increments) doesn't deadlock; patched to the
                    # real threshold post-scheduling, pre-compile.
                    rdma_gates.append(nc.tensor.wait_ge(gt_rsem, 0))
                    if gtdbg_ap is not None:
                        dbg_gate = nc.sync.wait_ge(gt_rsem, 0)
                        rdma_gates.append(dbg_gate)
                        nc.sync.dma_start(out=gtdbg_ap, in_=gt8_sb[:])
                elif SHARD_A:
                    # Gathered blocks land partition-concatenated by core:
                    # ag_dst[c] holds G.T rows [c*LS*128, (c+LS)*128), which
                    # map onto the LS adjacent (contiguous) es slices of
                    # gt8_sb starting at c*LS.
                    for c in range(N_CORES):
                        _dma_engs[c % 3].dma_start(
                            out=gt8_sb[:, c * LS:(c + 1) * LS, :],
                            in_=ag_dst[c])
                else:
                    for chunk in _chunks(range(KT), GROUP):
                        group = []
                        for dt_i in chunk:
                            wt_t = wt_pool.tile([128, KT, 128], FP8, tag="wt",
                                                name=f"wt{dt_i}")
                            nc.sync.dma_start(out=wt_t[:], in_=wt_ap[dt_i])
                            pg = psum_pool.tile([128, M], F32, tag="s")
                            group.append((dt_i, wt_t, pg))
                        for k in range(PT):
                            ks = slice(2 * k, 2 * k + 2)
                            for dt_i, wt_t, pg in group:
                                for j0, j1 in _j_ranges():
                                    js = slice(j0, j1)
                                    nc.tensor.matmul(pg[:, js], wt_t[:, ks, :],
                                                     st8_sb[:, ks, js],
                                                     perf_mode=DR,
                                                     start=(k == 0),
                                                     stop=(k == PT - 1))
                        for dt_i, wt_t, pg in group:
                            nc.vector.tensor_scalar_mul(gt8_sb[:, dt_i, :],
                                                        pg[:], 1.0 / W_SCALE)

                # Epilogue math: logits = lu/se, then sum softplus(-logits).
                # softplus(-x) = ln(z), z = 1 + exp(-x), initial guess
                # relu(-x) + ln2*exp(-0.7213*|x|), then Newton steps
                # y <- y - 1 + z*exp(-y). Stays within the Exp/Relu/Copy table.
                # Elementwise ops go to GpSimd (DVE is busy with phase C and is
                # strict FIFO); the row-sum uses the ACT accumulator. Half 0 is
                # emitted mid-phase-C so its serial chain overlaps; only two
                # tiny matmuls run at the very end.
                LN2 = 0.6931471805599453
                NH = NT // 2
                parts = []

                def epilogue_half(h, ve, nsteps=2):
                    hs = slice(h * NH, (h + 1) * NH)

                    def ht(name):
                        return singles.tile([128, NH], F32, name=f"{name}_h{h}")

                    rse = ht("rse")
                    nc.vector.reciprocal(rse[:], se_buf[:, hs])
                    lg = ht("lg")
                    ve.tensor_mul(lg[:], lu_buf[:, hs], rse[:])
                    emx = ht("emx")
                    nc.scalar.activation(out=emx[:], in_=lg[:], func=Exp,
                                         scale=-1.0)
                    z_t = ht("z_t")
                    ve.tensor_scalar_add(z_t[:], emx[:], 1.0)
                    rneg = ht("rneg")
                    nc.scalar.activation(out=rneg[:], in_=lg[:], func=Relu,
                                         scale=-1.0)
                    rpos = ht("rpos")
                    nc.scalar.activation(out=rpos[:], in_=lg[:], func=Relu,
                                         scale=1.0)
                    absx = ht("absx")
                    ve.tensor_add(absx[:], rneg[:], rpos[:])
                    g0 = ht("g0")
                    nc.scalar.activation(out=g0[:], in_=absx[:], func=Exp,
                                         scale=-0.7213)
                    y_t = ht("y0")
                    ve.tensor_scalar(out=y_t[:], in0=g0[:], scalar1=LN2,
                                     scalar2=None, op0=mybir.AluOpType.mult)
                    ve.tensor_add(y_t[:], y_t[:], rneg[:])
                    for step in range(nsteps):
                        e_n = ht(f"e_n{step}")
                        nc.scalar.activation(out=e_n[:], in_=y_t[:], func=Exp,
                                             scale=-1.0)
                        t_n = ht(f"t_n{step}")
                        ve.tensor_mul(t_n[:], z_t[:], e_n[:])
                        y2 = ht(f"y2_{step}")
                        ve.tensor_scalar(out=y2[:], in0=t_n[:], scalar1=-1.0,
                                         scalar2=None, op0=mybir.AluOpType.add)
                        ve.tensor_add(y2[:], y2[:], y_t[:])
                        y_t = y2
                    part = ht("part")
                    ysc = ht("ysc")
                    # free-dim row-sum via the ACT accumulator (gpsimd can only
                    # reduce along partitions; DVE would block phase C's queue)
                    nc.scalar.activation(
                        out=ysc[:], in_=y_t[:],
                        func=mybir.ActivationFunctionType.Identity,
                        accum_out=part[:, 0:1])
                    parts.append(part)

                # ---- Phase C: B = D @ G.T, lu = rowsum(e * B), fp8 DR ----
                for chunk in _chunks(range(NT), GROUP):
                    group = []
                    for it in chunk:
                        pb = psum_pool.tile([128, M], F32, tag="s")
                        group.append((it, dts[it], pb))
                    for k in range(PT):
                        ks = slice(2 * k, 2 * k + 2)
                        for it, dt_t, pb in group:
                            for j0, j1 in _j_ranges():
                                js = slice(j0, j1)
                                nc.tensor.matmul(pb[:, js], dt_t[:, ks, :],
                                                 gt8_sb[:, ks, js],
                                                 perf_mode=DR, start=(k == 0),
                                                 stop=(k == PT - 1))
                    for it, dt_t, pb in group:
                        # PSUM fp32 reads put DVE in 1x mode (2.3us/tile, above
                        # PE's matmul pace); cast B to bf16 on ACT first so the
                        # DVE mul and reduce run in 2-byte fast modes.
                        # (Measured: beats both DVE-direct-from-PSUM variants.)
                        b16_t = prod_pool.tile([128, M], BF16, tag="b16")
                        nc.scalar.copy(b16_t[:], pb[:])
                        prod_t = prod_pool.tile([128, M], BF16, tag="p")
                        nc.vector.tensor_mul(prod_t[:], b16_t[:],
                                             e_all[:, it, :])
                        nc.vector.reduce_sum(lu_buf[:, it:it + 1], prod_t[:],
                                             mybir.AxisListType.X)
                        if it == NH - 1:
                            epilogue_half(0, nc.gpsimd)
                epilogue_half(1, nc.vector, nsteps=1)

                ones_t = singles.tile([128, 1], F32)
                nc.vector.memset(ones_t[:], 1.0)
                tot = psum_pool.tile([128, M], F32, tag="s")
                for h in range(2):
                    nc.tensor.matmul(tot[0:1, 0:1], parts[h][:, 0:1], ones_t[:],
                                     start=(h == 0), stop=(h == 1))
                out_sb = singles.tile([1, 1], F32)
                nc.scalar.mul(out_sb[:], tot[0:1, 0:1], -1.0)
                nc.sync.dma_start(out=out_ap, in_=out_sb[:])

            if LOOP_R and LOOP_R > 1:
                if SHARD_A is True:
                    # Prime the gathered G.T once: the collective cannot run
                    # inside a hardware loop. Inputs are constant across
                    # iterations, so the primed ag_dst equals what a per-
                    # iteration gather would produce; the loop body still does
                    # all per-iteration compute, the src write and the
                    # gather-in reads.
                    st8_pr = singles.tile([128, KT, M], FP8, name="st8_prime")
                    for es in range(KT):
                        (nc.sync, nc.scalar, nc.gpsimd)[es % 3].dma_start(
                            out=st8_pr[:, es, :], in_=st8_ap[:, es, :])
                    gt_pr = singles.tile([128, LS, M], FP8, name="gt_prime")
                    _emit_phase_a_local(st8_pr, gt_pr, emit_cc=True)
                with tc.For_i(0, LOOP_R, 1):
                    _emit_body(emit_cc=False)
            else:
                _emit_body(emit_cc=True)

    if SHARD_A == "rdma":
        # Give the arrival gates their real thresholds now that scheduling is
        # done (the single-core scheduling sim would deadlock on a wait whose
        # increments only ever come from remote cores). In loop builds the
        # absolute threshold is enforced by iteration 1 and trivially true
        # after; the transport itself still runs every iteration. Inputs are
        # constant across loop iterations, so late remote re-writes of slots
        # 1-7 are value-identical and benign.
        want = int(os.environ.get('RDMA_GATE', 2 * (N_CORES - 1)))
        n_patched = 0
        for g in rdma_gates:
            for w in g.ins.sync_info.on_wait:
                if w.ant_name == "gt_rsem":
                    w.wait_value = want
                    n_patched += 1
        assert n_patched >= 1, "rdma gate patch found no gt_rsem wait"
    nc.compile()
    return nc


_NC_CACHE = None


def _get_program():
    global _NC_CACHE
    if _NC_CACHE is None:
        _NC_CACHE = _build_program()
    return _NC_CACHE


def _pack_inputs(D, S, W):
    """Host-side shard + transpose-pack + fp8 cast. Returns per-core maps."""
    f8 = ml_dtypes.float8_e4m3
    D8 = D.astype(f8)
    # stp8[p, es, j] = S[j, es*128+p]
    stp8 = np.ascontiguousarray(
        S.astype(f8).reshape(M, KT, 128).transpose(2, 1, 0))
    # wtp[dt, p, es, ii] = 64*W[dt*128+ii, es*128+p]
    W64 = (W.astype(np.float32) * W_SCALE).astype(f8)
    wtp = np.ascontiguousarray(
        W64.reshape(KT, 128, KT, 128).transpose(0, 3, 2, 1))
    in_maps = []
    for c in range(N_CORES):
        Dc = D8[c * N_LOC:(c + 1) * N_LOC]
        # dtp[it, p, ds, ii] = D_shard[it*128+ii, ds*128+p]
        dtp = np.ascontiguousarray(
            Dc.reshape(NT, 128, KT, 128).transpose(0, 3, 2, 1))
        if SHARD_A == "rdma":
            # Permute contraction-slice blocks so that local slot d holds
            # global block perm[d] — absorbing the remote_dma relative-dest
            # mapping so every core's program uses identical slot indices.
            # Contraction sums are permutation-invariant as long as dtp,
            # stp8 and wtp's es axis share the same order.
            perm = _block_perm(c)
            order = [b * LS + s for b in perm for s in range(LS)]
            dtp = np.ascontiguousarray(dtp[:, :, order, :])
            stp8_c = np.ascontiguousarray(stp8[:, order, :])
            wtp_c = np.ascontiguousarray(
                wtp[c * LS:(c + 1) * LS][:, :, order, :])
        else:
            stp8_c = stp8
            wtp_c = (np.ascontiguousarray(wtp[c * LS:(c + 1) * LS])
                     if SHARD_A else wtp)
        in_maps.append({"dtp": dtp, "stp8": stp8_c, "wtp": wtp_c})
    return in_maps


def kernel(D: np.ndarray, S: np.ndarray, W: np.ndarray) -> np.ndarray:
    assert D.shape == (N_FULL, DS) and S.shape == (M, DS) and W.shape == (DS, DS)
    nc = _get_program()
    in_maps = _pack_inputs(np.asarray(D), np.asarray(S), np.asarray(W))
    res = run_bass_kernel_spmd(nc, in_maps, core_ids=list(range(N_CORES)))
    total = np.float64(0.0)
    for r in res.results:
        total += np.float64(r["out"][0, 0])
    return np.array(total, dtype=np.float32)
